# revision 12
# baseline (speedup 1.0000x reference)
"""Trainium2 Bass kernel for nn_EpisodicMemory (scatter_memory).

Reference semantics (per batch row b):
    q        = s @ Wq.T
    att      = (q . mem_k[b,s,:]) / sqrt(D)        -> softmax over s -> w_read
    read_out = sum_s w_read[s] * mem_v[b,s,:]
    logits   = s @ Wl.T + bl ; top-8 -> sparse softmax weights w at idx
    mem_k2   = mem_k + gate * w * (write_vec @ Wk.T - mem_k)   (only 8 rows/batch change)
    mem_v2   = mem_v + gate * w * (write_vec @ Wv.T - mem_v)
    write_strength = gate[:, 0]

Strategy: pure data-parallel over batch on 8 NeuronCores (8 rows each).
Per core the kernel streams mem_k/mem_v through SBUF once (read+write =
the bulk of the ~128MB/core traffic), computing attention on the fly, and
patches the 64 top-k-modified rows afterwards with indirect DMA
gather/compute/scatter.
"""

import sys

sys.path.insert(0, "/opt/trn_rl_repo")

import numpy as np

from concourse import bacc, bass, mybir, tile
from concourse.masks import make_identity

F32 = mybir.dt.float32
I32 = mybir.dt.int32
U32 = mybir.dt.uint32

B, S, D, NCORES, TOPK = 64, 2048, 512, 8, 8
P = 128
Copy = mybir.ActivationFunctionType.Copy
Exp = mybir.ActivationFunctionType.Exp


def build_nc(BL=B // NCORES, S=S, D=D, stream_bufs=3):
    """Build the per-core Bass program. All cores run the same program (SPMD)."""
    ST = S // P          # s-subtiles per batch inside one [128, ST*D] tile
    KD = D // P          # K-tiles over the contraction dim D
    NBW = min(512, S)    # logits N-block width
    NB = S // NBW        # N-blocks for the logits matmul
    assert S % P == 0 and D % P == 0
    scale = 1.0 / float(np.sqrt(D))

    nc = bacc.Bacc("TRN2", target_bir_lowering=False, debug=False,
                   num_devices=NCORES)

    s_d = nc.dram_tensor("s", [BL, D], F32, kind="ExternalInput")
    wvec_d = nc.dram_tensor("write_vec", [BL, D], F32, kind="ExternalInput")
    gate_d = nc.dram_tensor("gate", [BL, 1], F32, kind="ExternalInput")
    memk_d = nc.dram_tensor("mem_k", [BL, S, D], F32, kind="ExternalInput")
    memv_d = nc.dram_tensor("mem_v", [BL, S, D], F32, kind="ExternalInput")
    Wq_d = nc.dram_tensor("Wq", [D, D], F32, kind="ExternalInput")
    Wl_d = nc.dram_tensor("Wl", [S, D], F32, kind="ExternalInput")
    bl_d = nc.dram_tensor("bl", [S], F32, kind="ExternalInput")
    Wk_d = nc.dram_tensor("Wk", [D, D], F32, kind="ExternalInput")
    Wv_d = nc.dram_tensor("Wv", [D, D], F32, kind="ExternalInput")

    ro_d = nc.dram_tensor("read_out", [BL, D], F32, kind="ExternalOutput")
    memk2_d = nc.dram_tensor("mem_k2", [BL, S, D], F32, kind="ExternalOutput")
    memv2_d = nc.dram_tensor("mem_v2", [BL, S, D], F32, kind="ExternalOutput")
    ws_d = nc.dram_tensor("write_strength", [BL], F32, kind="ExternalOutput")

    NROWS = BL * TOPK  # rows patched per core (64)

    with tile.TileContext(nc) as tc:
        with (
            tc.tile_pool(name="const", bufs=1) as cpool,
            tc.tile_pool(name="small", bufs=1) as sm,
            tc.tile_pool(name="smx", bufs=2) as smx,
            tc.tile_pool(name="perb", bufs=1) as perb,
            tc.tile_pool(name="q128p", bufs=2) as q128p,
            tc.tile_pool(name="prod", bufs=2) as prodp,
            tc.tile_pool(name="rows", bufs=1) as rows,
            tc.tile_pool(name="dram", bufs=1, space="DRAM") as dram,
            tc.tile_pool(name="ps_t", bufs=2, space="PSUM") as ps_t,
            tc.tile_pool(name="ps_proj", bufs=1, space="PSUM") as ps_proj,
            tc.tile_pool(name="ps_den", bufs=1, space="PSUM") as ps_den,
            tc.tile_pool(name="ps_ro", bufs=2, space="PSUM") as ps_ro,
            tc.tile_pool(name="ps_a", bufs=1, space="PSUM") as ps_a,
        ):
            # ---- constants / small loads ----
            ident = cpool.tile([P, P], F32)
            make_identity(nc, ident[:])
            ones_1xP = cpool.tile([1, P], F32)
            nc.vector.memset(ones_1xP[:], 1.0)
            ones_Px1 = cpool.tile([P, 1], F32)
            nc.vector.memset(ones_Px1[:], 1.0)

            s_sb = cpool.tile([BL, D], F32)
            nc.sync.dma_start(out=s_sb[:], in_=s_d[:])
            wvec_sb = cpool.tile([BL, D], F32)
            nc.sync.dma_start(out=wvec_sb[:], in_=wvec_d[:])
            gate_sb = cpool.tile([BL, 1], F32)
            nc.sync.dma_start(out=gate_sb[:], in_=gate_d[:])
            bl_sb = cpool.tile([1, S], F32)
            nc.sync.dma_start(out=bl_sb[:], in_=bl_d[0:S, None].rearrange("a b -> b a"))

            # ---- transpose s and write_vec: sT_k, wvT_k [128, BL] per K-tile ----
            def transpose_rows(x_sb, prefix):
                out = []
                for k in range(KD):
                    pt = ps_t.tile([P, P], F32, tag="ps_t", name=f"pt_{prefix}{k}")
                    nc.tensor.transpose(out=pt[:, :BL],
                                        in_=x_sb[:, k * P:(k + 1) * P],
                                        identity=ident[:BL, :BL])
                    t = sm.tile([P, BL], F32, tag=f"{prefix}{k}", name=f"{prefix}{k}")
                    nc.scalar.activation(out=t[:], in_=pt[:, :BL], func=Copy)
                    out.append(t)
                return out

            sT = transpose_rows(s_sb, "sT")
            wvT = transpose_rows(wvec_sb, "wvT")

            q_sb = sm.tile([BL, D], F32, tag="q_sb")
            wk_sb = sm.tile([BL, D], F32, tag="wk_sb")
            wv_sb = sm.tile([BL, D], F32, tag="wv_sb")
            lg_sb = sm.tile([BL, S], F32, tag="lg_sb")

            # ---- scoped: weight staging + transposes; freed before streaming ----
            with (
                tc.tile_pool(name="wstage", bufs=2) as wstage,
                tc.tile_pool(name="wT", bufs=1) as wT,
                tc.tile_pool(name="wlT", bufs=1) as wlT,
            ):
                def load_transposed(w_d, n_rows, pool, prefix):
                    # w_d: [n_rows, D] torch-Linear weight; tiles[k] = [128, n_rows]
                    # holding w_d[:, kP:(k+1)P].T (contraction dim D on partitions)
                    tiles = [pool.tile([P, n_rows], F32, tag=f"{prefix}{k}",
                                       name=f"{prefix}{k}") for k in range(KD)]
                    for r in range(n_rows // P):
                        st = wstage.tile([P, D], F32, tag="wstage", name=f"st_{prefix}{r}")
                        nc.sync.dma_start(out=st[:], in_=w_d[r * P:(r + 1) * P, :])
                        for k in range(KD):
                            pt = ps_t.tile([P, P], F32, tag="ps_t",
                                           name=f"pt_{prefix}{r}_{k}")
                            nc.tensor.transpose(out=pt[:],
                                                in_=st[:, k * P:(k + 1) * P],
                                                identity=ident[:])
                            nc.scalar.activation(
                                out=tiles[k][:, r * P:(r + 1) * P], in_=pt[:],
                                func=Copy)
                    return tiles

                WqT = load_transposed(Wq_d, D, wT, "WqT")
                WkT = load_transposed(Wk_d, D, wT, "WkT")
                WvT = load_transposed(Wv_d, D, wT, "WvT")
                WlT = load_transposed(Wl_d, S, wlT, "WlT")

                # ---- q / write_k / write_v projections: x @ W.T ----
                def project(xT, WT, o, post_scale=1.0):
                    pq = ps_proj.tile([P, max(D, NBW)], F32, tag="ps_proj",
                                      name=f"pq_{o.tensor.name}")
                    for k in range(KD):
                        nc.tensor.matmul(out=pq[:BL, :D], lhsT=xT[k][:], rhs=WT[k][:],
                                         start=(k == 0), stop=(k == KD - 1))
                    if post_scale == 1.0:
                        nc.scalar.activation(out=o[:], in_=pq[:BL, :D], func=Copy)
                    else:
                        nc.scalar.activation(out=o[:], in_=pq[:BL, :D], func=Copy,
                                             scale=float(post_scale))

                # fold 1/sqrt(D) into q so attention logits come out pre-scaled
                project(sT, WqT, q_sb, post_scale=scale)
                project(wvT, WkT, wk_sb)
                project(wvT, WvT, wv_sb)

                # ---- write logits = s @ Wl.T + bl ----
                for n in range(NB):
                    sl = slice(n * NBW, (n + 1) * NBW)
                    pq = ps_proj.tile([P, max(D, NBW)], F32, tag="ps_proj",
                                      name=f"pl_{n}")
                    for k in range(KD):
                        nc.tensor.matmul(out=pq[:BL, :NBW], lhsT=sT[k][:],
                                         rhs=WlT[k][:, sl], start=(k == 0), stop=False)
                    # bias via K=1 matmul with a ones row
                    nc.tensor.matmul(out=pq[:BL, :NBW], lhsT=ones_1xP[:, :BL],
                                     rhs=bl_sb[:, sl], start=False, stop=True)
                    nc.scalar.activation(out=lg_sb[:, sl], in_=pq[:BL, :NBW],
                                         func=Copy)

            # ---- top-8 of logits ----
            vals = sm.tile([BL, 8], F32, tag="vals")
            nc.vector.max(out=vals[:], in_=lg_sb[:])
            idx_u = sm.tile([BL, 8], U32, tag="idx_u")
            nc.vector.max_index(out=idx_u[:], in_max=vals[:], in_values=lg_sb[:])

            # sparse softmax over the 8 vals (vals[:,0] is the rowwise max)
            negmax = sm.tile([BL, 1], F32, tag="negmax")
            nc.vector.tensor_scalar_mul(negmax[:], vals[:, 0:1], -1.0)
            e8 = sm.tile([BL, 8], F32, tag="e8")
            sum8 = sm.tile([BL, 1], F32, tag="sum8")
            nc.scalar.activation(out=e8[:], in_=vals[:], func=Exp,
                                 bias=negmax[:], accum_out=sum8[:])
            inv8 = sm.tile([BL, 1], F32, tag="inv8")
            nc.vector.reciprocal(inv8[:], sum8[:])
            wsp = sm.tile([BL, 8], F32, tag="wsp")
            nc.vector.tensor_scalar_mul(wsp[:], e8[:], inv8[:])
            a8 = sm.tile([BL, 8], F32, tag="a8")
            nc.vector.tensor_scalar_mul(a8[:], wsp[:], gate_sb[:])

            # ---- global row ids r = b*S + idx  (f32 arithmetic, exact) ----
            base_i = sm.tile([BL, 8], I32, tag="base_i")
            nc.gpsimd.iota(base_i[:], pattern=[[0, 8]], base=0, channel_multiplier=S)
            base_f = sm.tile([BL, 8], F32, tag="base_f")
            nc.vector.tensor_copy(base_f[:], base_i[:])
            idx_f = sm.tile([BL, 8], F32, tag="idx_f")
            nc.vector.tensor_copy(idx_f[:], idx_u[:])
            r_f = sm.tile([BL, 8], F32, tag="r_f")
            nc.vector.tensor_add(r_f[:], base_f[:], idx_f[:])
            r_u = sm.tile([BL, 8], U32, tag="r_u")
            nc.vector.tensor_copy(r_u[:], r_f[:])

            # flatten [BL,8] -> [64,1] via a DRAM bounce
            r_dram = dram.tile([BL, 8], U32, tag="r_dram")
            nc.sync.dma_start(out=r_dram[:], in_=r_u[:])
            r64 = rows.tile([NROWS, 1], U32, tag="r64")
            nc.sync.dma_start(
                out=r64[:],
                in_=r_dram[:].rearrange("a b -> (a b)")[0:NROWS, None])
            a_dram = dram.tile([BL, 8], F32, tag="a_dram")
            nc.sync.dma_start(out=a_dram[:], in_=a8[:])
            a64 = rows.tile([NROWS, 1], F32, tag="a64")
            nc.sync.dma_start(
                out=a64[:],
                in_=a_dram[:].rearrange("a b -> (a b)")[0:NROWS, None])

            # ---- expand write_k/write_v to one row per patched slot ----
            # E[b, r] = 1 iff r // TOPK == b ; then w64 = E.T @ write_k
            eio = sm.tile([BL, NROWS], I32, tag="eio")
            nc.gpsimd.iota(eio[:], pattern=[[1, NROWS]], base=0,
                           channel_multiplier=-TOPK)
            eiof = sm.tile([BL, NROWS], F32, tag="eiof")
            nc.vector.tensor_copy(eiof[:], eio[:])
            m_ge = sm.tile([BL, NROWS], F32, tag="m_ge")
            nc.vector.tensor_scalar(m_ge[:], eiof[:], -0.5, None,
                                    op0=mybir.AluOpType.is_ge)
            m_le = sm.tile([BL, NROWS], F32, tag="m_le")
            nc.vector.tensor_scalar(m_le[:], eiof[:], TOPK - 0.5, None,
                                    op0=mybir.AluOpType.is_le)
            emat = sm.tile([BL, NROWS], F32, tag="emat")
            nc.vector.tensor_mul(emat[:], m_ge[:], m_le[:])

            wk64 = rows.tile([NROWS, D], F32, tag="wk64")
            wv64 = rows.tile([NROWS, D], F32, tag="wv64")
            pk = ps_a.tile([NROWS, D], F32, tag="ps_a", name="pk")
            nc.tensor.matmul(out=pk[:], lhsT=emat[:], rhs=wk_sb[:],
                             start=True, stop=True)
            nc.scalar.activation(out=wk64[:], in_=pk[:], func=Copy)
            pv = ps_a.tile([NROWS, D], F32, tag="ps_a", name="pv")
            nc.tensor.matmul(out=pv[:], lhsT=emat[:], rhs=wv_sb[:],
                             start=True, stop=True)
            nc.scalar.activation(out=wv64[:], in_=pv[:], func=Copy)

            # ---- gather original rows, compute patched rows ----
            memk_flat = memk_d[:].rearrange("b s d -> (b s) d")
            memv_flat = memv_d[:].rearrange("b s d -> (b s) d")
            krows = rows.tile([NROWS, D], F32, tag="krows")
            nc.gpsimd.indirect_dma_start(
                out=krows[:], out_offset=None, in_=memk_flat,
                in_offset=bass.IndirectOffsetOnAxis(ap=r64[:, :1], axis=0))
            vrows = rows.tile([NROWS, D], F32, tag="vrows")
            nc.gpsimd.indirect_dma_start(
                out=vrows[:], out_offset=None, in_=memv_flat,
                in_offset=bass.IndirectOffsetOnAxis(ap=r64[:, :1], axis=0))

            def patch_rows(orig, w64, tag):
                dlt = rows.tile([NROWS, D], F32, tag=f"dlt_{tag}",
                                name=f"dlt_{tag}")
                nc.vector.tensor_sub(dlt[:], w64[:], orig[:])
                nc.vector.tensor_scalar_mul(dlt[:], dlt[:], a64[:])
                out_t = rows.tile([NROWS, D], F32, tag=f"patched_{tag}",
                                  name=f"patched_{tag}")
                nc.vector.tensor_add(out_t[:], orig[:], dlt[:])
                return out_t

            ck = patch_rows(krows, wk64, "k")
            cv = patch_rows(vrows, wv64, "v")

            # ---- broadcast q rows across all 128 partitions (startup) ----
            # q128_all[:, b*D:(b+1)*D] = q[b] replicated; via K=1 ones-matmul.
            q128_all = cpool.tile([P, BL * D], F32)
            for b in range(BL):
                qrow = q128p.tile([1, D], F32, tag="qrow", name=f"qrow{b}")
                nc.sync.dma_start(out=qrow[:], in_=q_sb[b:b + 1, :])
                for k in range(KD):
                    ksl = slice(k * P, (k + 1) * P)
                    qp = ps_den.tile([P, P], F32, tag="ps_q128",
                                     name=f"qp{b}_{k}")
                    nc.tensor.matmul(out=qp[:], lhsT=ones_1xP[:],
                                     rhs=qrow[:, ksl], start=True, stop=True)
                    nc.scalar.activation(out=q128_all[:, b * D + k * P:
                                                      b * D + (k + 1) * P],
                                         in_=qp[:], func=Copy)

            # ---- stream mem_k / mem_v; attention on the fly ----
            FW = ST * D  # free width of one whole-batch tile
            attT = [perb.tile([P, ST], F32, tag=f"attT{b}", name=f"attT{b}")
                    for b in range(BL)]
            u16 = [perb.tile([P, ST], F32, tag=f"u16_{b}", name=f"u16_{b}")
                   for b in range(BL)]
            inv_d = [perb.tile([1, 1], F32, tag=f"inv_{b}", name=f"inv_{b}")
                     for b in range(BL)]

            k_stores = []
            v_stores = []

            with tc.tile_pool(name="stream", bufs=stream_bufs) as stream:
                for b in range(BL):
                    # --- phase A: stream mem_k[b], compute attT_b ---
                    kt = stream.tile([P, FW], F32, tag="stream", name=f"kt{b}")
                    nc.sync.dma_start(
                        out=kt[:],
                        in_=memk_d[b].rearrange("(p t) d -> p (t d)", p=P))
                    q128 = q128_all[:, b * D:(b + 1) * D]
                    for j in range(ST):
                        pr = prodp.tile([P, D], F32, tag="prod", name=f"pr{b}_{j}")
                        nc.vector.tensor_mul(pr[:], kt[:, j * D:(j + 1) * D], q128)
                        nc.vector.reduce_sum(out=attT[b][:, j:j + 1], in_=pr[:],
                                             axis=mybir.AxisListType.X)
                    st_i = nc.scalar.dma_start(
                        out=memk2_d[b].rearrange("(p t) d -> p (t d)", p=P),
                        in_=kt[:])
                    k_stores.append(st_i)

                    # --- phase B: two-level softmax (unnormalized; 1/denom
                    # folded into the read_out copy) ---
                    m1 = smx.tile([P, 1], F32, tag="m1", name=f"m1_{b}")
                    nc.vector.reduce_max(out=m1[:], in_=attT[b][:],
                                         axis=mybir.AxisListType.X)
                    negm1 = smx.tile([P, 1], F32, tag="negm1", name=f"negm1_{b}")
                    nc.vector.tensor_scalar_mul(negm1[:], m1[:], -1.0)
                    e1 = smx.tile([P, ST], F32, tag="e1", name=f"e1_{b}")
                    s1 = smx.tile([P, 1], F32, tag="s1", name=f"s1_{b}")
                    nc.scalar.activation(out=e1[:], in_=attT[b][:], func=Exp,
                                         bias=negm1[:], accum_out=s1[:])
                    texp = smx.tile([P, 1], F32, tag="texp", name=f"texp_{b}")
                    nc.scalar.activation(out=texp[:], in_=m1[:], func=Exp)
                    nc.vector.tensor_scalar_mul(u16[b][:], e1[:], texp[:])
                    contrib = smx.tile([P, 1], F32, tag="contrib",
                                       name=f"contrib_{b}")
                    nc.vector.tensor_mul(contrib[:], s1[:], texp[:])
                    dps = ps_den.tile([1, 1], F32, tag="ps_den", name=f"dps{b}")
                    nc.tensor.matmul(out=dps[:], lhsT=contrib[:], rhs=ones_Px1[:],
                                     start=True, stop=True)
                    nc.vector.reciprocal(inv_d[b][:], dps[:])

                    # --- phase C: stream mem_v[b], accumulate read_out ---
                    vt = stream.tile([P, FW], F32, tag="stream", name=f"vt{b}")
                    nc.sync.dma_start(
                        out=vt[:],
                        in_=memv_d[b].rearrange("(p t) d -> p (t d)", p=P))
                    rp = ps_ro.tile([1, D], F32, tag="ps_ro", name=f"rp{b}")
                    for j in range(ST):
                        nc.tensor.matmul(out=rp[:], lhsT=u16[b][:, j:j + 1],
                                         rhs=vt[:, j * D:(j + 1) * D],
                                         start=(j == 0), stop=(j == ST - 1))
                    ro_row = q128p.tile([1, D], F32, tag="ro_row",
                                        name=f"ro_row{b}")
                    nc.scalar.activation(out=ro_row[:], in_=rp[:],
                                         func=Copy, scale=inv_d[b][:])
                    nc.sync.dma_start(out=ro_d[b:b + 1, :], in_=ro_row[:])
                    st_i = nc.scalar.dma_start(
                        out=memv2_d[b].rearrange("(p t) d -> p (t d)", p=P),
                        in_=vt[:])
                    v_stores.append(st_i)

                # ---- outputs ----
                nc.sync.dma_start(out=ws_d[0:BL, None], in_=gate_sb[:])

                # ---- scatter patched rows (after the bulk stores) ----
                memk2_flat = memk2_d[:].rearrange("b s d -> (b s) d")
                memv2_flat = memv2_d[:].rearrange("b s d -> (b s) d")
                sc_k = nc.gpsimd.indirect_dma_start(
                    out=memk2_flat,
                    out_offset=bass.IndirectOffsetOnAxis(ap=r64[:, :1], axis=0),
                    in_=ck[:], in_offset=None)
                sc_v = nc.gpsimd.indirect_dma_start(
                    out=memv2_flat,
                    out_offset=bass.IndirectOffsetOnAxis(ap=r64[:, :1], axis=0),
                    in_=cv[:], in_offset=None)
                for st_i in k_stores:
                    tile.add_dep_helper(sc_k.ins, st_i.ins, sync=True,
                                        reason="patch rows after bulk mem_k2 store")
                for st_i in v_stores:
                    tile.add_dep_helper(sc_v.ins, st_i.ins, sync=True,
                                        reason="patch rows after bulk mem_v2 store")

    nc.compile()
    return nc


_NC_CACHE = {}


def _get_nc():
    key = (B, S, D)
    if key not in _NC_CACHE:
        _NC_CACHE[key] = build_nc()
    return _NC_CACHE[key]


def kernel(s, write_vec, mem_k, mem_v, gate, Wq, Wl, bl, Wk, Wv, topk):
    assert int(topk) == TOPK
    s = np.ascontiguousarray(np.asarray(s, dtype=np.float32))
    write_vec = np.ascontiguousarray(np.asarray(write_vec, dtype=np.float32))
    mem_k = np.ascontiguousarray(np.asarray(mem_k, dtype=np.float32))
    mem_v = np.ascontiguousarray(np.asarray(mem_v, dtype=np.float32))
    gate = np.ascontiguousarray(np.asarray(gate, dtype=np.float32))
    Wq = np.ascontiguousarray(np.asarray(Wq, dtype=np.float32))
    Wl = np.ascontiguousarray(np.asarray(Wl, dtype=np.float32))
    bl = np.ascontiguousarray(np.asarray(bl, dtype=np.float32))
    Wk = np.ascontiguousarray(np.asarray(Wk, dtype=np.float32))
    Wv = np.ascontiguousarray(np.asarray(Wv, dtype=np.float32))

    from concourse.bass_utils import run_bass_kernel_spmd

    nc = _get_nc()
    BL = B // NCORES
    in_maps = []
    for c in range(NCORES):
        sl = slice(c * BL, (c + 1) * BL)
        in_maps.append({
            "s": s[sl], "write_vec": write_vec[sl], "gate": gate[sl],
            "mem_k": mem_k[sl], "mem_v": mem_v[sl],
            "Wq": Wq, "Wl": Wl, "bl": bl, "Wk": Wk, "Wv": Wv,
        })
    res = run_bass_kernel_spmd(nc, in_maps, list(range(NCORES)))
    ro = np.concatenate([res.results[c]["read_out"] for c in range(NCORES)], axis=0)
    mk2 = np.concatenate([res.results[c]["mem_k2"] for c in range(NCORES)], axis=0)
    mv2 = np.concatenate([res.results[c]["mem_v2"] for c in range(NCORES)], axis=0)
    ws = np.concatenate([res.results[c]["write_strength"] for c in range(NCORES)],
                        axis=0)
    return ro, mk2, mv2, ws
